# revision 5
# baseline (speedup 1.0000x reference)
"""GAT (2-layer, PyG-style) on 8 Trainium2 NeuronCores via Bass/Tile.

Strategy (edge/node-parallel hybrid):
  - Host (integer-only preprocessing): append self loops, sort edges by dst,
    shard dst nodes across 8 cores (2500 each), build per-core chunk schedules
    (chunks of 128 edges, each chunk's dsts within one 128-node window).
  - Launch A (8 cores): replicated dense phase h=[x@W1 | a_src | a_dst] for all
    nodes -> DRAM table; then per-core aggregation over owned dst windows:
    indirect-DMA gather of src rows + dst attention rows, segment softmax
    (no max-subtraction needed: logits are O(5), exp is safe in fp32) via
    one-hot scatter matmul accumulating [num | denom] in PSUM; epilogue
    divides, biases, ELUs, and computes conv2's per-node [h2_pre|a_src2|a_dst2].
  - Host: concat per-core outputs into the conv2 gather table (data movement).
  - Launch B (8 cores): conv2 aggregation (1 head, 32 ch) same scheme + global
    mean-pool partials per graph via one-hot matmul.
  - Launch C (1 core): sum partials, scale by 1/count, FC layer.
"""

import numpy as np
from contextlib import ExitStack

import concourse.bass as bass
import concourse.bacc as bacc
import concourse.mybir as mybir
import concourse.tile as tile
from concourse.bass import IndirectOffsetOnAxis
from concourse.bass_utils import run_bass_kernel_spmd
from concourse.masks import make_identity
from concourse.bass import _add_dep_helper as _add_dep

P = 128
N_NODES = 20000
NCORES = 8
NPC = N_NODES // NCORES  # 2500 nodes per core
F_IN = 128
HID = 32
HEADS = 8
HH = HEADS * HID  # 256
NGRAPH = 64
NCLS = 40
NWIN = (NPC + P - 1) // P  # 20 windows per core (19 full + 68)

f32 = mybir.dt.float32
i32 = mybir.dt.int32
FT = mybir.ActivationFunctionType
OP = mybir.AluOpType

_cache = {}


# ---------------------------------------------------------------- host prep
def _host_prep(edge_index, batch):
    src = np.concatenate([edge_index[0], np.arange(N_NODES)]).astype(np.int64)
    dst = np.concatenate([edge_index[1], np.arange(N_NODES)]).astype(np.int64)
    order = np.argsort(dst, kind="stable")
    src, dst = src[order], dst[order]

    # per-core, per-window edge lists
    counts = np.zeros((NCORES, NWIN), dtype=np.int64)
    # window id of each edge (global): dst -> core k = dst//2500, w = (dst%2500)//128
    core_of = dst // NPC
    win_of = (dst % NPC) // P
    for k in range(NCORES):
        m = core_of == k
        counts[k] = np.bincount(win_of[m], minlength=NWIN)
    K = np.maximum(1, (counts + P - 1) // P).max(axis=0)  # chunks per window, shared
    nchunk = int(K.sum())
    cbase = np.zeros(NWIN, dtype=np.int64)
    cbase[1:] = np.cumsum(K)[:-1]

    SRCT = np.zeros((NCORES, P, nchunk), dtype=np.int32)
    DSTIT = np.zeros((NCORES, P, nchunk), dtype=np.int32)
    DSTLT = np.full((NCORES, P, nchunk), 999.0, dtype=np.float32)
    for k in range(NCORES):
        m = core_of == k
        s_k, d_k, w_k = src[m], dst[m], win_of[m]
        for w in range(NWIN):
            wm = w_k == w
            s_w, d_w = s_k[wm], d_k[wm]
            n = len(s_w)
            nch = (n + P - 1) // P if n else 0
            for j in range(nch):
                lo, hi = j * P, min((j + 1) * P, n)
                c = cbase[w] + j
                SRCT[k, : hi - lo, c] = s_w[lo:hi]
                DSTIT[k, : hi - lo, c] = d_w[lo:hi]
                DSTLT[k, : hi - lo, c] = (d_w[lo:hi] - (k * NPC + w * P)).astype(
                    np.float32
                )

    batch = np.asarray(batch).astype(np.int64)
    GON = np.zeros((NCORES, P, NWIN * NGRAPH), dtype=np.float32)
    for k in range(NCORES):
        for w in range(NWIN):
            base = k * NPC + w * P
            wn = min(P, NPC - w * P)
            for p in range(wn):
                GON[k, p, w * NGRAPH + batch[base + p]] = 1.0
    cnt = np.bincount(batch, minlength=NGRAPH).astype(np.float32)
    INVC = (1.0 / np.maximum(cnt, 1.0)).reshape(NGRAPH, 1).astype(np.float32)
    return (K.tolist(), nchunk, cbase.tolist(), SRCT, DSTIT, DSTLT, GON, INVC)


def _elu(nc, sb, he_out, h1, tag):
    """he_out = elu(h1) = max(h1,0) + exp(min(h1,0)) - 1. h1/he_out: [P, W] sbuf."""
    w = h1.shape[-1]
    neg = sb.tile([P, w], dtype=f32, tag=f"{tag}neg")
    nc.vector.tensor_scalar(out=neg[:], in0=h1, scalar1=0.0, scalar2=None, op0=OP.min)
    enx = sb.tile([P, w], dtype=f32, tag=f"{tag}enx")
    nc.scalar.activation(enx[:], neg[:], FT.Exp)
    pos = sb.tile([P, w], dtype=f32, tag=f"{tag}pos")
    nc.vector.tensor_scalar(out=pos[:], in0=h1, scalar1=0.0, scalar2=None, op0=OP.max)
    nc.vector.tensor_tensor(out=he_out, in0=enx[:], in1=pos[:], op=OP.add)
    nc.vector.tensor_scalar(
        out=he_out, in0=he_out, scalar1=1.0, scalar2=None, op0=OP.subtract
    )


# ---------------------------------------------------------------- launch A
def _build_A(K, nchunk, cbase):
    nc = bacc.Bacc("TRN2", target_bir_lowering=False, debug=False,
                   num_devices=NCORES)
    x_d = nc.dram_tensor("x", [N_NODES, F_IN], f32, kind="ExternalInput")
    w1_d = nc.dram_tensor("w1", [F_IN, HH], f32, kind="ExternalInput")
    acat_d = nc.dram_tensor("acat", [P, 2, 16], f32, kind="ExternalInput")
    iota_d = nc.dram_tensor("iota", [P, P], f32, kind="ExternalInput")
    b1b_d = nc.dram_tensor("b1b", [P, HH], f32, kind="ExternalInput")
    w2r_d = nc.dram_tensor("w2r", [P, 2 * HID], f32, kind="ExternalInput")
    att2_d = nc.dram_tensor("att2", [HID, 2], f32, kind="ExternalInput")
    srct_d = nc.dram_tensor("srct", [P, nchunk], i32, kind="ExternalInput")
    dstit_d = nc.dram_tensor("dstit", [P, nchunk], i32, kind="ExternalInput")
    dstlt_d = nc.dram_tensor("dstlt", [P, nchunk], f32, kind="ExternalInput")
    outa_d = nc.dram_tensor("outA", [NPC, 34], f32, kind="ExternalOutput")

    hplus_d = nc.dram_tensor("hplus", [N_NODES, HH + 8], f32)
    adst_d = nc.dram_tensor("adst", [N_NODES, 8], f32)

    NT = (N_NODES + P - 1) // P  # 157 node tiles (last = 32 rows)

    with tile.TileContext(nc, num_cores=NCORES) as tc, ExitStack() as ctx:
        const = ctx.enter_context(tc.tile_pool(name="const", bufs=1))
        ident = const.tile([P, P], dtype=f32)
        make_identity(nc, ident[:])
        iota_sb = const.tile([P, P], dtype=f32)
        nc.sync.dma_start(out=iota_sb[:], in_=iota_d[:, :])
        b1b_sb = const.tile([P, HH], dtype=f32)
        nc.sync.dma_start(out=b1b_sb[:], in_=b1b_d[:, :])
        w2r_sb = const.tile([P, 2 * HID], dtype=f32)
        nc.sync.dma_start(out=w2r_sb[:], in_=w2r_d[:, :])
        att2_sb = const.tile([HID, 2], dtype=f32)
        nc.sync.dma_start(out=att2_sb[:], in_=att2_d[:, :])
        srct_sb = const.tile([P, nchunk], dtype=i32)
        nc.sync.dma_start(out=srct_sb[:], in_=srct_d[:, :])
        dstit_sb = const.tile([P, nchunk], dtype=i32)
        nc.sync.dma_start(out=dstit_sb[:], in_=dstit_d[:, :])
        dstlt_sb = const.tile([P, nchunk], dtype=f32)
        nc.sync.dma_start(out=dstlt_sb[:], in_=dstlt_d[:, :])

        # ---- one-time: W1ext = [W1 | W1 @ Acat]  (Acat: blockdiag att1)
        w1ext = const.tile([P, HH + 16], dtype=f32)
        nc.sync.dma_start(out=w1ext[:, 0:HH], in_=w1_d[:, :])
        acat_sb = const.tile([P, 2, 16], dtype=f32)
        nc.sync.dma_start(out=acat_sb[:], in_=acat_d[:, :, :])
        store_insts = []
        with tc.tile_pool(name="psinit", bufs=2, space="PSUM") as psinit, \
             tc.tile_pool(name="sbinit", bufs=2) as sbinit:
            w1t = []
            for hf in range(2):
                tp = psinit.tile([P, P], dtype=f32, tag="tp")
                nc.tensor.transpose(tp[:], w1ext[:, hf * P : (hf + 1) * P], ident[:])
                w1th = sbinit.tile([P, P], dtype=f32, tag="w1t")
                nc.scalar.copy(w1th[:], tp[:])
                w1t.append(w1th)
            w1aps = psinit.tile([P, 16], dtype=f32, tag="w1a")
            for hf in range(2):
                nc.tensor.matmul(
                    out=w1aps[:], lhsT=w1t[hf][:], rhs=acat_sb[:, hf, :],
                    start=(hf == 0), stop=(hf == 1),
                )
            nc.scalar.copy(w1ext[:, HH : HH + 16], w1aps[:])

            # ---- dense phase: hplus = [x@W1 | a_src], adst = a_dst (all nodes)
            for i in range(NT):
                rows = min(P, N_NODES - i * P)
                xt = sbinit.tile([P, F_IN], dtype=f32, tag="xt")
                nc.sync.dma_start(out=xt[:rows], in_=x_d[i * P : i * P + rows, :])
                tp = psinit.tile([P, P], dtype=f32, tag="tp")
                nc.tensor.transpose(tp[:], xt[:], ident[:])
                xT = sbinit.tile([P, P], dtype=f32, tag="xT")
                nc.scalar.copy(xT[:], tp[:])
                hps = psinit.tile([P, HH + 16], dtype=f32, tag="hps")
                nc.tensor.matmul(out=hps[:], lhsT=xT[:], rhs=w1ext[:],
                                 start=True, stop=True)
                hsb = sbinit.tile([P, HH + 16], dtype=f32, tag="hsb")
                nc.vector.tensor_copy(hsb[:], hps[:])
                s1 = nc.sync.dma_start(
                    out=hplus_d[i * P : i * P + rows, :], in_=hsb[:rows, 0 : HH + 8]
                )
                s2 = nc.sync.dma_start(
                    out=adst_d[i * P : i * P + rows, :],
                    in_=hsb[:rows, HH + 8 : HH + 16],
                )
                store_insts.extend([s1, s2])

        # ---- aggregation over owned windows
        sb = ctx.enter_context(tc.tile_pool(name="agg", bufs=8))
        sbs = ctx.enter_context(tc.tile_pool(name="aggs", bufs=8))
        sbe = ctx.enter_context(tc.tile_pool(name="epi", bufs=3))
        pswin = ctx.enter_context(tc.tile_pool(name="pswin", bufs=2, space="PSUM"))
        pstp = ctx.enter_context(tc.tile_pool(name="pstp", bufs=2, space="PSUM"))
        pssm = ctx.enter_context(tc.tile_pool(name="pssm", bufs=2, space="PSUM"))

        for w in range(NWIN):
            wn = min(P, NPC - w * P)
            win_ps = pswin.tile([P, HH + 8], dtype=f32, tag="win")
            kw = K[w]
            for j in range(kw):
                c = cbase[w] + j
                g = sb.tile([P, HH + 8], dtype=f32, tag="g")
                gi = nc.gpsimd.indirect_dma_start(
                    out=g[:], out_offset=None, in_=hplus_d[:, :],
                    in_offset=IndirectOffsetOnAxis(ap=srct_sb[:, c : c + 1], axis=0),
                )
                ad = sbs.tile([P, 8], dtype=f32, tag="ad")
                ai = nc.gpsimd.indirect_dma_start(
                    out=ad[:], out_offset=None, in_=adst_d[:, :],
                    in_offset=IndirectOffsetOnAxis(ap=dstit_sb[:, c : c + 1], axis=0),
                )
                for st in store_insts:
                    _add_dep(gi.ins, st.ins, sync=True, reason="table RAW")
                    _add_dep(ai.ins, st.ins, sync=True, reason="table RAW")
                S = sb.tile([P, P], dtype=f32, tag="S")
                nc.vector.tensor_tensor(
                    out=S[:], in0=dstlt_sb[:, c : c + 1].to_broadcast([P, P]),
                    in1=iota_sb[:], op=OP.is_equal,
                )
                e8 = sbs.tile([P, 8], dtype=f32, tag="e8")
                nc.vector.tensor_tensor(
                    out=e8[:], in0=g[:, HH : HH + 8], in1=ad[:], op=OP.add
                )
                el = sbs.tile([P, 8], dtype=f32, tag="el")
                nc.scalar.activation(el[:], e8[:], FT.Prelu, alpha=0.2)
                V = sb.tile([P, HH + 8], dtype=f32, tag="V")
                nc.scalar.activation(V[:, HH : HH + 8], el[:], FT.Exp)
                nc.vector.tensor_tensor(
                    out=V[:, 0:HH].rearrange("p (h c) -> p h c", h=HEADS),
                    in0=g[:, 0:HH].rearrange("p (h c) -> p h c", h=HEADS),
                    in1=V[:, HH : HH + 8].to_broadcast([P, HEADS, HID]),
                    op=OP.mult,
                )
                nc.tensor.matmul(
                    out=win_ps[:], lhsT=S[:], rhs=V[:],
                    start=(j == 0), stop=(j == kw - 1),
                )

            # epilogue: h1 = elu(num/den + b1); h2pre/a2 for conv2
            den = sbe.tile([P, 8], dtype=f32, tag="den")
            nc.vector.tensor_scalar(
                out=den[:], in0=win_ps[:, HH : HH + 8], scalar1=1e-30,
                scalar2=None, op0=OP.max,
            )
            rec = sbe.tile([P, 8], dtype=f32, tag="rec")
            nc.vector.reciprocal(rec[:], den[:])
            h1 = sbe.tile([P, HH], dtype=f32, tag="h1")
            nc.vector.tensor_tensor(
                out=h1[:].rearrange("p (h c) -> p h c", h=HEADS),
                in0=win_ps[:, 0:HH].rearrange("p (h c) -> p h c", h=HEADS),
                in1=rec[:].to_broadcast([P, HEADS, HID]),
                op=OP.mult,
            )
            nc.vector.tensor_tensor(out=h1[:], in0=h1[:], in1=b1b_sb[:], op=OP.add)
            he = sbe.tile([P, HH], dtype=f32, tag="he")
            _elu(nc, sbe, he[:], h1[:], "e1")
            # h2pre = he @ W2  (contraction over 256 via 2 transposes)
            h2ps = pssm.tile([P, HID], dtype=f32, tag="small")
            for hf in range(2):
                tp = pstp.tile([P, P], dtype=f32, tag="tp")
                nc.tensor.transpose(tp[:], he[:, hf * P : (hf + 1) * P], ident[:])
                hT = sbe.tile([P, P], dtype=f32, tag="hT")
                nc.scalar.copy(hT[:], tp[:])
                nc.tensor.matmul(
                    out=h2ps[:], lhsT=hT[:], rhs=w2r_sb[:, hf * HID : (hf + 1) * HID],
                    start=(hf == 0), stop=(hf == 1),
                )
            outw = sbe.tile([P, 34], dtype=f32, tag="outw")
            nc.scalar.copy(outw[:, 0:HID], h2ps[:])
            t3 = pstp.tile([P, P], dtype=f32, tag="tp")
            nc.tensor.transpose(t3[0:HID, :], outw[:, 0:HID], ident[:])
            h2T = sbe.tile([HID, P], dtype=f32, tag="h2T")
            nc.scalar.copy(h2T[:], t3[0:HID, :])
            a2ps = pssm.tile([P, 2], dtype=f32, tag="small")
            nc.tensor.matmul(out=a2ps[:], lhsT=h2T[:], rhs=att2_sb[:],
                             start=True, stop=True)
            nc.scalar.copy(outw[:, 32:34], a2ps[:])
            nc.sync.dma_start(
                out=outa_d[w * P : w * P + wn, :], in_=outw[:wn, :]
            )
    nc.compile()
    return nc


# ---------------------------------------------------------------- launch B
def _build_B(K, nchunk, cbase):
    nc = bacc.Bacc("TRN2", target_bir_lowering=False, debug=False,
                   num_devices=NCORES)
    hp2_d = nc.dram_tensor("hp2", [N_NODES, 33], f32, kind="ExternalInput")
    ad2_d = nc.dram_tensor("ad2", [N_NODES, 1], f32, kind="ExternalInput")
    iota_d = nc.dram_tensor("iota", [P, P], f32, kind="ExternalInput")
    b2b_d = nc.dram_tensor("b2b", [P, HID], f32, kind="ExternalInput")
    gon_d = nc.dram_tensor("gon", [P, NWIN * NGRAPH], f32, kind="ExternalInput")
    srct_d = nc.dram_tensor("srct", [P, nchunk], i32, kind="ExternalInput")
    dstit_d = nc.dram_tensor("dstit", [P, nchunk], i32, kind="ExternalInput")
    dstlt_d = nc.dram_tensor("dstlt", [P, nchunk], f32, kind="ExternalInput")
    outb_d = nc.dram_tensor("outB", [NGRAPH, HID], f32, kind="ExternalOutput")

    with tile.TileContext(nc, num_cores=NCORES) as tc, ExitStack() as ctx:
        const = ctx.enter_context(tc.tile_pool(name="const", bufs=1))
        iota_sb = const.tile([P, P], dtype=f32)
        nc.sync.dma_start(out=iota_sb[:], in_=iota_d[:, :])
        b2b_sb = const.tile([P, HID], dtype=f32)
        nc.sync.dma_start(out=b2b_sb[:], in_=b2b_d[:, :])
        gon_sb = const.tile([P, NWIN * NGRAPH], dtype=f32)
        nc.sync.dma_start(out=gon_sb[:], in_=gon_d[:, :])
        srct_sb = const.tile([P, nchunk], dtype=i32)
        nc.sync.dma_start(out=srct_sb[:], in_=srct_d[:, :])
        dstit_sb = const.tile([P, nchunk], dtype=i32)
        nc.sync.dma_start(out=dstit_sb[:], in_=dstit_d[:, :])
        dstlt_sb = const.tile([P, nchunk], dtype=f32)
        nc.sync.dma_start(out=dstlt_sb[:], in_=dstlt_d[:, :])

        sb = ctx.enter_context(tc.tile_pool(name="agg", bufs=8))
        sbs = ctx.enter_context(tc.tile_pool(name="aggs", bufs=8))
        sbe = ctx.enter_context(tc.tile_pool(name="epi", bufs=3))
        pswin = ctx.enter_context(tc.tile_pool(name="pswin", bufs=2, space="PSUM"))
        pspool = ctx.enter_context(tc.tile_pool(name="pspool", bufs=1, space="PSUM"))
        pool_ps = pspool.tile([NGRAPH, HID], dtype=f32)

        for w in range(NWIN):
            wn = min(P, NPC - w * P)
            win_ps = pswin.tile([P, 33], dtype=f32, tag="win")
            kw = K[w]
            for j in range(kw):
                c = cbase[w] + j
                g = sb.tile([P, 33], dtype=f32, tag="g")
                nc.gpsimd.indirect_dma_start(
                    out=g[:], out_offset=None, in_=hp2_d[:, :],
                    in_offset=IndirectOffsetOnAxis(ap=srct_sb[:, c : c + 1], axis=0),
                )
                ad = sbs.tile([P, 1], dtype=f32, tag="ad")
                nc.gpsimd.indirect_dma_start(
                    out=ad[:], out_offset=None, in_=ad2_d[:, :],
                    in_offset=IndirectOffsetOnAxis(ap=dstit_sb[:, c : c + 1], axis=0),
                )
                S = sb.tile([P, P], dtype=f32, tag="S")
                nc.vector.tensor_tensor(
                    out=S[:], in0=dstlt_sb[:, c : c + 1].to_broadcast([P, P]),
                    in1=iota_sb[:], op=OP.is_equal,
                )
                e1 = sbs.tile([P, 1], dtype=f32, tag="e1")
                nc.vector.tensor_tensor(
                    out=e1[:], in0=g[:, 32:33], in1=ad[:], op=OP.add
                )
                el = sbs.tile([P, 1], dtype=f32, tag="el")
                nc.scalar.activation(el[:], e1[:], FT.Prelu, alpha=0.2)
                V = sb.tile([P, 33], dtype=f32, tag="V")
                nc.scalar.activation(V[:, 32:33], el[:], FT.Exp)
                nc.vector.tensor_tensor(
                    out=V[:, 0:HID], in0=g[:, 0:HID],
                    in1=V[:, 32:33].to_broadcast([P, HID]), op=OP.mult,
                )
                nc.tensor.matmul(
                    out=win_ps[:], lhsT=S[:], rhs=V[:],
                    start=(j == 0), stop=(j == kw - 1),
                )

            den = sbe.tile([P, 1], dtype=f32, tag="den")
            nc.vector.tensor_scalar(
                out=den[:], in0=win_ps[:, 32:33], scalar1=1e-30,
                scalar2=None, op0=OP.max,
            )
            rec = sbe.tile([P, 1], dtype=f32, tag="rec")
            nc.vector.reciprocal(rec[:], den[:])
            h2 = sbe.tile([P, HID], dtype=f32, tag="h2")
            nc.vector.tensor_tensor(
                out=h2[:], in0=win_ps[:, 0:HID],
                in1=rec[:].to_broadcast([P, HID]), op=OP.mult,
            )
            nc.vector.tensor_tensor(out=h2[:], in0=h2[:], in1=b2b_sb[:], op=OP.add)
            he = sbe.tile([P, HID], dtype=f32, tag="he")
            _elu(nc, sbe, he[:], h2[:], "e2")
            nc.tensor.matmul(
                out=pool_ps[:], lhsT=gon_sb[:, w * NGRAPH : (w + 1) * NGRAPH],
                rhs=he[:], start=(w == 0), stop=(w == NWIN - 1),
            )
        po = const.tile([NGRAPH, HID], dtype=f32)
        nc.vector.tensor_copy(po[:], pool_ps[:])
        nc.sync.dma_start(out=outb_d[:, :], in_=po[:, :])
    nc.compile()
    return nc


# ---------------------------------------------------------------- launch C
def _build_C():
    nc = bacc.Bacc("TRN2", target_bir_lowering=False, debug=False, num_devices=1)
    part_d = nc.dram_tensor("part", [NGRAPH, NCORES * HID], f32, kind="ExternalInput")
    invc_d = nc.dram_tensor("invc", [NGRAPH, 1], f32, kind="ExternalInput")
    fcw_d = nc.dram_tensor("fcw", [HID, NCLS], f32, kind="ExternalInput")
    fcbb_d = nc.dram_tensor("fcbb", [NGRAPH, NCLS], f32, kind="ExternalInput")
    out_d = nc.dram_tensor("out", [NGRAPH, NCLS], f32, kind="ExternalOutput")

    with tile.TileContext(nc, num_cores=1) as tc, ExitStack() as ctx:
        sb = ctx.enter_context(tc.tile_pool(name="sb", bufs=1))
        ps = ctx.enter_context(tc.tile_pool(name="ps", bufs=1, space="PSUM"))
        ident = sb.tile([NGRAPH, NGRAPH], dtype=f32)
        make_identity(nc, ident[:])
        pa = sb.tile([NGRAPH, NCORES * HID], dtype=f32)
        nc.sync.dma_start(out=pa[:], in_=part_d[:, :])
        invc = sb.tile([NGRAPH, 1], dtype=f32)
        nc.sync.dma_start(out=invc[:], in_=invc_d[:, :])
        fcw = sb.tile([HID, NCLS], dtype=f32)
        nc.sync.dma_start(out=fcw[:], in_=fcw_d[:, :])
        fcbb = sb.tile([NGRAPH, NCLS], dtype=f32)
        nc.sync.dma_start(out=fcbb[:], in_=fcbb_d[:, :])

        acc = sb.tile([NGRAPH, HID], dtype=f32)
        nc.vector.tensor_copy(acc[:], pa[:, 0:HID])
        for k in range(1, NCORES):
            nc.vector.tensor_tensor(
                out=acc[:], in0=acc[:], in1=pa[:, k * HID : (k + 1) * HID], op=OP.add
            )
        nc.vector.tensor_tensor(
            out=acc[:], in0=acc[:], in1=invc[:].to_broadcast([NGRAPH, HID]),
            op=OP.mult,
        )
        tps = ps.tile([HID, NGRAPH], dtype=f32)
        nc.tensor.transpose(tps[:], acc[:], ident[:])
        gT = sb.tile([HID, NGRAPH], dtype=f32)
        nc.scalar.copy(gT[:], tps[:])
        ops = ps.tile([NGRAPH, NCLS], dtype=f32)
        nc.tensor.matmul(out=ops[:], lhsT=gT[:], rhs=fcw[:], start=True, stop=True)
        osb = sb.tile([NGRAPH, NCLS], dtype=f32)
        nc.vector.tensor_tensor(out=osb[:], in0=ops[:], in1=fcbb[:], op=OP.add)
        nc.sync.dma_start(out=out_d[:, :], in_=osb[:, :])
    nc.compile()
    return nc


# ---------------------------------------------------------------- driver
def _run(inputs, trace=False):
    x = np.ascontiguousarray(np.asarray(inputs["x"], dtype=np.float32))
    edge_index = np.asarray(inputs["edge_index"], dtype=np.int64)
    batch = np.asarray(inputs["batch"], dtype=np.int64)
    W1 = np.ascontiguousarray(np.asarray(inputs["W1"], dtype=np.float32))
    att_src1 = np.asarray(inputs["att_src1"], dtype=np.float32)
    att_dst1 = np.asarray(inputs["att_dst1"], dtype=np.float32)
    b1 = np.asarray(inputs["b1"], dtype=np.float32)
    W2 = np.ascontiguousarray(np.asarray(inputs["W2"], dtype=np.float32))
    att_src2 = np.asarray(inputs["att_src2"], dtype=np.float32)
    att_dst2 = np.asarray(inputs["att_dst2"], dtype=np.float32)
    b2 = np.asarray(inputs["b2"], dtype=np.float32)
    fc_w = np.ascontiguousarray(np.asarray(inputs["fc_w"], dtype=np.float32))
    fc_b = np.asarray(inputs["fc_b"], dtype=np.float32)

    key = (edge_index.tobytes(), batch.tobytes())
    hkey = hash(key)
    if hkey not in _cache:
        prep = _host_prep(edge_index, batch)
        K, nchunk, cbase = prep[0], prep[1], prep[2]
        ncA = _build_A(K, nchunk, cbase)
        ncB = _build_B(K, nchunk, cbase)
        ncC = _build_C()
        _cache.clear()
        _cache[hkey] = (prep, ncA, ncB, ncC)
    prep, ncA, ncB, ncC = _cache[hkey]
    K, nchunk, cbase, SRCT, DSTIT, DSTLT, GON, INVC = prep

    # layout-only host tensors
    acat = np.zeros((2, P, 16), dtype=np.float32)  # [halfrow, f, 16]
    for h in range(HEADS):
        for c in range(HID):
            gidx = h * HID + c
            acat[gidx // P, gidx % P, h] = att_src1[h, c]
            acat[gidx // P, gidx % P, 8 + h] = att_dst1[h, c]
    acat = np.ascontiguousarray(acat.transpose(1, 0, 2))  # [P, 2, 16]
    iota = np.tile(np.arange(P, dtype=np.float32), (P, 1))
    b1b = np.tile(b1.reshape(1, HH), (P, 1)).astype(np.float32)
    w2r = np.concatenate([W2[0:P, :], W2[P : 2 * P, :]], axis=1)  # [128, 64]
    att2 = np.stack([att_src2[0], att_dst2[0]], axis=1).astype(np.float32)  # [32,2]
    b2b = np.tile(b2.reshape(1, HID), (P, 1)).astype(np.float32)
    fcbb = np.tile(fc_b.reshape(1, NCLS), (NGRAPH, 1)).astype(np.float32)

    in_maps_A = [
        {
            "x": x, "w1": W1, "acat": acat, "iota": iota, "b1b": b1b,
            "w2r": w2r, "att2": att2,
            "srct": np.ascontiguousarray(SRCT[k]),
            "dstit": np.ascontiguousarray(DSTIT[k]),
            "dstlt": np.ascontiguousarray(DSTLT[k]),
        }
        for k in range(NCORES)
    ]
    resA = run_bass_kernel_spmd(ncA, in_maps_A, list(range(NCORES)), trace=trace)
    outAs = [resA.results[k]["outA"] for k in range(NCORES)]
    hp2 = np.ascontiguousarray(np.concatenate([o[:, 0:33] for o in outAs], axis=0))
    ad2 = np.ascontiguousarray(np.concatenate([o[:, 33:34] for o in outAs], axis=0))

    in_maps_B = [
        {
            "hp2": hp2, "ad2": ad2, "iota": iota, "b2b": b2b,
            "gon": np.ascontiguousarray(GON[k]),
            "srct": np.ascontiguousarray(SRCT[k]),
            "dstit": np.ascontiguousarray(DSTIT[k]),
            "dstlt": np.ascontiguousarray(DSTLT[k]),
        }
        for k in range(NCORES)
    ]
    resB = run_bass_kernel_spmd(ncB, in_maps_B, list(range(NCORES)), trace=trace)
    part = np.concatenate(
        [resB.results[k]["outB"] for k in range(NCORES)], axis=1
    )  # [64, 8*32]
    in_map_C = {
        "part": np.ascontiguousarray(part), "invc": INVC, "fcw": fc_w, "fcbb": fcbb,
    }
    resC = run_bass_kernel_spmd(ncC, [in_map_C], [0], trace=trace)
    out = resC.results[0]["out"]
    return out, (resA, resB, resC)


def kernel(**inputs) -> np.ndarray:
    out, _ = _run(inputs, trace=False)
    return out


def run_profiled(inputs):
    return _run(inputs, trace=True)


# revision 10
# speedup vs baseline: 1.5935x; 1.5935x over previous
"""GAT (2-layer, PyG-style) on 8 Trainium2 NeuronCores via Bass/Tile.

Strategy (edge/node-parallel hybrid):
  - Host (integer-only preprocessing): append self loops, sort edges by dst,
    shard dst nodes across 8 cores (2500 each), build per-core chunk schedules
    (chunks of 128 edges, each chunk's dsts within one 128-node window).
  - Launch A (8 cores): replicated dense phase h=[x@W1 | a_src | a_dst] for all
    nodes -> DRAM table; then per-core aggregation over owned dst windows:
    indirect-DMA gather of src rows + dst attention rows, segment softmax
    (no max-subtraction needed: logits are O(5), exp is safe in fp32) via
    one-hot scatter matmul accumulating [num | denom] in PSUM; epilogue
    divides, biases, ELUs, and computes conv2's per-node [h2_pre|a_src2|a_dst2].
  - Host: concat per-core outputs into the conv2 gather table (data movement).
  - Launch B (8 cores): conv2 aggregation (1 head, 32 ch) same scheme + global
    mean-pool partials per graph via one-hot matmul.
  - Launch C (1 core): sum partials, scale by 1/count, FC layer.
"""

import numpy as np
import ml_dtypes
from contextlib import ExitStack

import concourse.bass as bass
import concourse.bacc as bacc
import concourse.mybir as mybir
import concourse.tile as tile
from concourse.bass import IndirectOffsetOnAxis
from concourse.bass_utils import run_bass_kernel_spmd
from concourse.masks import make_identity
from concourse.bass import _add_dep_helper as _add_dep

P = 128
N_NODES = 20000
NCORES = 8
NPC = N_NODES // NCORES  # 2500 nodes per core
F_IN = 128
HID = 32
HEADS = 8
HH = HEADS * HID  # 256
NGRAPH = 64
NCLS = 40
NWIN = (NPC + P - 1) // P  # 20 windows per core (19 full + 68)

f32 = mybir.dt.float32
bf16 = mybir.dt.bfloat16
i32 = mybir.dt.int32
FT = mybir.ActivationFunctionType
OP = mybir.AluOpType

_cache = {}


# ---------------------------------------------------------------- host prep
def _host_prep(edge_index, batch):
    src = np.concatenate([edge_index[0], np.arange(N_NODES)]).astype(np.int64)
    dst = np.concatenate([edge_index[1], np.arange(N_NODES)]).astype(np.int64)
    order = np.argsort(dst, kind="stable")
    src, dst = src[order], dst[order]

    # per-core, per-window edge lists
    counts = np.zeros((NCORES, NWIN), dtype=np.int64)
    # window id of each edge (global): dst -> core k = dst//2500, w = (dst%2500)//128
    core_of = dst // NPC
    win_of = (dst % NPC) // P
    for k in range(NCORES):
        m = core_of == k
        counts[k] = np.bincount(win_of[m], minlength=NWIN)
    K = np.maximum(1, (counts + P - 1) // P).max(axis=0)  # chunks per window, shared
    nchunk = int(K.sum())
    cbase = np.zeros(NWIN, dtype=np.int64)
    cbase[1:] = np.cumsum(K)[:-1]

    SRCT = np.zeros((NCORES, P, nchunk), dtype=np.int32)
    DSTIT = np.zeros((NCORES, P, nchunk), dtype=np.int32)
    DSTLT = np.full((NCORES, P, nchunk), 999.0, dtype=np.float32)
    for k in range(NCORES):
        m = core_of == k
        s_k, d_k, w_k = src[m], dst[m], win_of[m]
        for w in range(NWIN):
            wm = w_k == w
            s_w, d_w = s_k[wm], d_k[wm]
            n = len(s_w)
            nch = (n + P - 1) // P if n else 0
            for j in range(nch):
                lo, hi = j * P, min((j + 1) * P, n)
                c = cbase[w] + j
                SRCT[k, : hi - lo, c] = s_w[lo:hi]
                DSTIT[k, : hi - lo, c] = d_w[lo:hi]
                DSTLT[k, : hi - lo, c] = (d_w[lo:hi] - (k * NPC + w * P)).astype(
                    np.float32
                )

    batch = np.asarray(batch).astype(np.int64)
    GON = np.zeros((NCORES, P, NWIN * NGRAPH), dtype=np.float32)
    for k in range(NCORES):
        for w in range(NWIN):
            base = k * NPC + w * P
            wn = min(P, NPC - w * P)
            for p in range(wn):
                GON[k, p, w * NGRAPH + batch[base + p]] = 1.0
    cnt = np.bincount(batch, minlength=NGRAPH).astype(np.float32)
    INVC = (1.0 / np.maximum(cnt, 1.0)).reshape(NGRAPH, 1).astype(np.float32)
    return (K.tolist(), nchunk, cbase.tolist(), SRCT, DSTIT, DSTLT, GON, INVC)


def _elu(nc, sb, he_out, h1, tag):
    """he_out = elu(h1) = max(h1,0) + exp(min(h1,0)) - 1. h1/he_out: [P, W] sbuf."""
    w = h1.shape[-1]
    neg = sb.tile([P, w], dtype=f32, tag=f"{tag}neg")
    nc.vector.tensor_scalar(out=neg[:], in0=h1, scalar1=0.0, scalar2=None, op0=OP.min)
    enx = sb.tile([P, w], dtype=f32, tag=f"{tag}enx")
    nc.scalar.activation(enx[:], neg[:], FT.Exp)
    pos = sb.tile([P, w], dtype=f32, tag=f"{tag}pos")
    nc.vector.tensor_scalar(out=pos[:], in0=h1, scalar1=0.0, scalar2=None, op0=OP.max)
    nc.vector.tensor_tensor(out=he_out, in0=enx[:], in1=pos[:], op=OP.add)
    nc.vector.tensor_scalar(
        out=he_out, in0=he_out, scalar1=1.0, scalar2=None, op0=OP.subtract
    )


# ---------------------------------------------------------------- launch A
def _build_A(K, nchunk, cbase):
    nc = bacc.Bacc("TRN2", target_bir_lowering=False, debug=False,
                   num_devices=NCORES)
    x_d = nc.dram_tensor("x", [N_NODES, F_IN], f32, kind="ExternalInput")
    w1_d = nc.dram_tensor("w1", [F_IN, HH], f32, kind="ExternalInput")
    acat_d = nc.dram_tensor("acat", [P, 2, 16], f32, kind="ExternalInput")
    iota_d = nc.dram_tensor("iota", [P, P], f32, kind="ExternalInput")
    b1b_d = nc.dram_tensor("b1b", [P, HH], f32, kind="ExternalInput")
    w2r_d = nc.dram_tensor("w2r", [P, 2 * HID], f32, kind="ExternalInput")
    att2_d = nc.dram_tensor("att2", [HID, 2], f32, kind="ExternalInput")
    srct_d = nc.dram_tensor("srct", [P, nchunk], i32, kind="ExternalInput")
    nodeidx_d = nc.dram_tensor("nodeidx", [P, NWIN], i32, kind="ExternalInput")
    dstlt_d = nc.dram_tensor("dstlt", [P, nchunk], f32, kind="ExternalInput")
    outa_d = nc.dram_tensor("outA", [NPC, 34], f32, kind="ExternalOutput")

    hplus_d = nc.dram_tensor("hplus", [N_NODES, HH + 8], bf16)
    adst_d = nc.dram_tensor("adst", [N_NODES, 8], bf16)

    NT = (N_NODES + P - 1) // P  # 157 node tiles (last = 32 rows)

    with tile.TileContext(nc, num_cores=NCORES) as tc, ExitStack() as ctx:
        const = ctx.enter_context(tc.tile_pool(name="const", bufs=1))
        ident = const.tile([P, P], dtype=f32)
        make_identity(nc, ident[:])
        iota_sb = const.tile([P, P], dtype=f32)
        nc.sync.dma_start(out=iota_sb[:], in_=iota_d[:, :])
        b1b_sb = const.tile([P, HH], dtype=f32)
        nc.sync.dma_start(out=b1b_sb[:], in_=b1b_d[:, :])
        w2r_sb = const.tile([P, 2 * HID], dtype=f32)
        nc.sync.dma_start(out=w2r_sb[:], in_=w2r_d[:, :])
        att2_sb = const.tile([HID, 2], dtype=f32)
        nc.sync.dma_start(out=att2_sb[:], in_=att2_d[:, :])
        srct_sb = const.tile([P, nchunk], dtype=i32)
        nc.sync.dma_start(out=srct_sb[:], in_=srct_d[:, :])
        nodeidx_sb = const.tile([P, NWIN], dtype=i32)
        nc.sync.dma_start(out=nodeidx_sb[:], in_=nodeidx_d[:, :])
        dstlt_sb = const.tile([P, nchunk], dtype=f32)
        nc.sync.dma_start(out=dstlt_sb[:], in_=dstlt_d[:, :])
        ident_bf = const.tile([P, P], dtype=bf16)
        make_identity(nc, ident_bf[:])
        w2r_bf = const.tile([P, 2 * HID], dtype=bf16)
        nc.vector.tensor_copy(w2r_bf[:], w2r_sb[:])

        # ---- one-time: W1ext = [W1 | W1 @ Acat]  (Acat: blockdiag att1)
        w1f = const.tile([P, HH + 16], dtype=f32)
        nc.sync.dma_start(out=w1f[:, 0:HH], in_=w1_d[:, :])
        w1ext = const.tile([P, HH + 16], dtype=bf16)
        acat_sb = const.tile([P, 2, 16], dtype=f32)
        nc.sync.dma_start(out=acat_sb[:], in_=acat_d[:, :, :])
        store_insts = []
        with tc.tile_pool(name="psinit", bufs=2, space="PSUM") as psinit, \
             tc.tile_pool(name="sbinit", bufs=2) as sbinit:
            w1t = []
            for hf in range(2):
                tp = psinit.tile([P, P], dtype=f32, tag="tp")
                nc.tensor.transpose(tp[:], w1f[:, hf * P : (hf + 1) * P], ident[:])
                w1th = sbinit.tile([P, P], dtype=f32, tag="w1t")
                nc.scalar.copy(w1th[:], tp[:])
                w1t.append(w1th)
            w1aps = psinit.tile([P, 16], dtype=f32, tag="w1a")
            for hf in range(2):
                nc.tensor.matmul(
                    out=w1aps[:], lhsT=w1t[hf][:], rhs=acat_sb[:, hf, :],
                    start=(hf == 0), stop=(hf == 1),
                )
            nc.scalar.copy(w1f[:, HH : HH + 16], w1aps[:])
            nc.vector.tensor_copy(w1ext[:], w1f[:])

            # ---- dense phase: hplus = [x@W1 | a_src], adst = a_dst (all nodes)
            for i in range(NT):
                rows = min(P, N_NODES - i * P)
                xt = sbinit.tile([P, F_IN], dtype=f32, tag="xt")
                nc.sync.dma_start(out=xt[:rows], in_=x_d[i * P : i * P + rows, :])
                tp = psinit.tile([P, P], dtype=f32, tag="tp")
                nc.tensor.transpose(tp[:], xt[:], ident[:])
                xT = sbinit.tile([P, P], dtype=bf16, tag="xT")
                nc.scalar.copy(xT[:], tp[:])
                hps = psinit.tile([P, HH + 16], dtype=f32, tag="hps")
                nc.tensor.matmul(out=hps[:], lhsT=xT[:], rhs=w1ext[:],
                                 start=True, stop=True)
                hsb = sbinit.tile([P, HH + 16], dtype=bf16, tag="hsb")
                nc.vector.tensor_copy(hsb[:], hps[:])
                s1 = nc.sync.dma_start(
                    out=hplus_d[i * P : i * P + rows, :], in_=hsb[:rows, 0 : HH + 8]
                )
                s2 = nc.sync.dma_start(
                    out=adst_d[i * P : i * P + rows, :],
                    in_=hsb[:rows, HH + 8 : HH + 16],
                )
                store_insts.extend([s1, s2])

        # ---- aggregation over owned windows
        sb = ctx.enter_context(tc.tile_pool(name="agg", bufs=8))
        sbs = ctx.enter_context(tc.tile_pool(name="aggs", bufs=8))
        sbe = ctx.enter_context(tc.tile_pool(name="epi", bufs=3))
        pswin = ctx.enter_context(tc.tile_pool(name="pswin", bufs=2, space="PSUM"))
        pstp = ctx.enter_context(tc.tile_pool(name="pstp", bufs=1, space="PSUM"))
        psstp = ctx.enter_context(tc.tile_pool(name="psstp", bufs=2, space="PSUM"))
        pssm = ctx.enter_context(tc.tile_pool(name="pssm", bufs=1, space="PSUM"))
        psed = ctx.enter_context(tc.tile_pool(name="psed", bufs=2, space="PSUM"))

        for w in range(NWIN):
            wn = min(P, NPC - w * P)
            win_ps = pswin.tile([P, HH + 8], dtype=f32, tag="win")
            adw = sbe.tile([P, 8], dtype=bf16, tag="adw")
            aw = nc.gpsimd.indirect_dma_start(
                out=adw[:], out_offset=None, in_=adst_d[:, :],
                in_offset=IndirectOffsetOnAxis(ap=nodeidx_sb[:, w : w + 1], axis=0),
            )
            for st in store_insts:
                _add_dep(aw.ins, st.ins, sync=True, reason="table RAW")
            kw = K[w]
            for j in range(kw):
                c = cbase[w] + j
                g = sb.tile([P, HH + 8], dtype=bf16, tag="g")
                gi = nc.gpsimd.indirect_dma_start(
                    out=g[:], out_offset=None, in_=hplus_d[:, :],
                    in_offset=IndirectOffsetOnAxis(ap=srct_sb[:, c : c + 1], axis=0),
                )
                for st in store_insts:
                    _add_dep(gi.ins, st.ins, sync=True, reason="table RAW")
                S = sb.tile([P, P], dtype=bf16, tag="S")
                nc.vector.tensor_tensor(
                    out=S[:], in0=dstlt_sb[:, c : c + 1].to_broadcast([P, P]),
                    in1=iota_sb[:], op=OP.is_equal,
                )
                stp = psstp.tile([P, P], dtype=bf16, tag="stp")
                nc.tensor.transpose(stp[:], S[:], ident_bf[:])
                St = sb.tile([P, P], dtype=bf16, tag="St")
                nc.vector.tensor_copy(St[:], stp[:])
                edp = psed.tile([P, 8], dtype=f32, tag="ed")
                nc.tensor.matmul(out=edp[:], lhsT=St[:], rhs=adw[:],
                                 start=True, stop=True)
                e8 = sbs.tile([P, 8], dtype=f32, tag="e8")
                nc.vector.tensor_tensor(
                    out=e8[:], in0=g[:, HH : HH + 8], in1=edp[:], op=OP.add
                )
                el = sbs.tile([P, 8], dtype=f32, tag="el")
                nc.scalar.activation(el[:], e8[:], FT.Prelu, alpha=0.2)
                V = sb.tile([P, HH + 8], dtype=bf16, tag="V")
                nc.scalar.activation(V[:, HH : HH + 8], el[:], FT.Exp)
                nc.vector.tensor_tensor(
                    out=V[:, 0:HH].rearrange("p (h c) -> p h c", h=HEADS),
                    in0=g[:, 0:HH].rearrange("p (h c) -> p h c", h=HEADS),
                    in1=V[:, HH : HH + 8].to_broadcast([P, HEADS, HID]),
                    op=OP.mult,
                )
                nc.tensor.matmul(
                    out=win_ps[:], lhsT=S[:], rhs=V[:],
                    start=(j == 0), stop=(j == kw - 1),
                )

            # epilogue: h1 = elu(num/den + b1); h2pre/a2 for conv2
            den = sbe.tile([P, 8], dtype=f32, tag="den")
            nc.vector.tensor_scalar(
                out=den[:], in0=win_ps[:, HH : HH + 8], scalar1=1e-30,
                scalar2=None, op0=OP.max,
            )
            rec = sbe.tile([P, 8], dtype=f32, tag="rec")
            nc.vector.reciprocal(rec[:], den[:])
            h1 = sbe.tile([P, HH], dtype=f32, tag="h1")
            nc.vector.tensor_tensor(
                out=h1[:].rearrange("p (h c) -> p h c", h=HEADS),
                in0=win_ps[:, 0:HH].rearrange("p (h c) -> p h c", h=HEADS),
                in1=rec[:].to_broadcast([P, HEADS, HID]),
                op=OP.mult,
            )
            nc.vector.tensor_tensor(out=h1[:], in0=h1[:], in1=b1b_sb[:], op=OP.add)
            he = sbe.tile([P, HH], dtype=f32, tag="he")
            _elu(nc, sbe, he[:], h1[:], "e1")
            # h2pre = he @ W2  (contraction over 256 via 2 transposes)
            h2ps = pssm.tile([P, HID], dtype=f32, tag="small")
            for hf in range(2):
                tp = pstp.tile([P, P], dtype=f32, tag="tp")
                nc.tensor.transpose(tp[:], he[:, hf * P : (hf + 1) * P], ident[:])
                hT = sbe.tile([P, P], dtype=bf16, tag="hT")
                nc.scalar.copy(hT[:], tp[:])
                nc.tensor.matmul(
                    out=h2ps[:], lhsT=hT[:], rhs=w2r_bf[:, hf * HID : (hf + 1) * HID],
                    start=(hf == 0), stop=(hf == 1),
                )
            outw = sbe.tile([P, 34], dtype=f32, tag="outw")
            nc.scalar.copy(outw[:, 0:HID], h2ps[:])
            t3 = pstp.tile([P, P], dtype=f32, tag="tp")
            nc.tensor.transpose(t3[0:HID, :], outw[:, 0:HID], ident[:])
            h2T = sbe.tile([HID, P], dtype=f32, tag="h2T")
            nc.scalar.copy(h2T[:], t3[0:HID, :])
            a2ps = pssm.tile([P, 2], dtype=f32, tag="small")
            nc.tensor.matmul(out=a2ps[:], lhsT=h2T[:], rhs=att2_sb[:],
                             start=True, stop=True)
            nc.scalar.copy(outw[:, 32:34], a2ps[:])
            nc.sync.dma_start(
                out=outa_d[w * P : w * P + wn, :], in_=outw[:wn, :]
            )
    nc.compile()
    return nc


# ---------------------------------------------------------------- launch B
def _build_B(K, nchunk, cbase):
    nc = bacc.Bacc("TRN2", target_bir_lowering=False, debug=False,
                   num_devices=NCORES)
    hp2_d = nc.dram_tensor("hp2", [N_NODES, 33], bf16, kind="ExternalInput")
    ad2_d = nc.dram_tensor("ad2", [N_NODES, 1], bf16, kind="ExternalInput")
    iota_d = nc.dram_tensor("iota", [P, P], f32, kind="ExternalInput")
    b2b_d = nc.dram_tensor("b2b", [P, HID], f32, kind="ExternalInput")
    gon_d = nc.dram_tensor("gon", [P, NWIN * NGRAPH], f32, kind="ExternalInput")
    srct_d = nc.dram_tensor("srct", [P, nchunk], i32, kind="ExternalInput")
    nodeidx_d = nc.dram_tensor("nodeidx", [P, NWIN], i32, kind="ExternalInput")
    dstlt_d = nc.dram_tensor("dstlt", [P, nchunk], f32, kind="ExternalInput")
    outb_d = nc.dram_tensor("outB", [NGRAPH, HID], f32, kind="ExternalOutput")

    with tile.TileContext(nc, num_cores=NCORES) as tc, ExitStack() as ctx:
        const = ctx.enter_context(tc.tile_pool(name="const", bufs=1))
        iota_sb = const.tile([P, P], dtype=f32)
        nc.sync.dma_start(out=iota_sb[:], in_=iota_d[:, :])
        b2b_sb = const.tile([P, HID], dtype=f32)
        nc.sync.dma_start(out=b2b_sb[:], in_=b2b_d[:, :])
        gon_sb = const.tile([P, NWIN * NGRAPH], dtype=f32)
        nc.sync.dma_start(out=gon_sb[:], in_=gon_d[:, :])
        srct_sb = const.tile([P, nchunk], dtype=i32)
        nc.sync.dma_start(out=srct_sb[:], in_=srct_d[:, :])
        nodeidx_sb = const.tile([P, NWIN], dtype=i32)
        nc.sync.dma_start(out=nodeidx_sb[:], in_=nodeidx_d[:, :])
        dstlt_sb = const.tile([P, nchunk], dtype=f32)
        nc.sync.dma_start(out=dstlt_sb[:], in_=dstlt_d[:, :])
        ident_bf = const.tile([P, P], dtype=bf16)
        make_identity(nc, ident_bf[:])

        sb = ctx.enter_context(tc.tile_pool(name="agg", bufs=8))
        sbs = ctx.enter_context(tc.tile_pool(name="aggs", bufs=8))
        sbe = ctx.enter_context(tc.tile_pool(name="epi", bufs=3))
        pswin = ctx.enter_context(tc.tile_pool(name="pswin", bufs=2, space="PSUM"))
        psstp = ctx.enter_context(tc.tile_pool(name="psstp", bufs=2, space="PSUM"))
        psed = ctx.enter_context(tc.tile_pool(name="psed", bufs=2, space="PSUM"))
        pspool = ctx.enter_context(tc.tile_pool(name="pspool", bufs=1, space="PSUM"))
        pool_ps = pspool.tile([NGRAPH, HID], dtype=f32)

        for w in range(NWIN):
            wn = min(P, NPC - w * P)
            win_ps = pswin.tile([P, 33], dtype=f32, tag="win")
            adw = sbe.tile([P, 1], dtype=bf16, tag="adw")
            nc.gpsimd.indirect_dma_start(
                out=adw[:], out_offset=None, in_=ad2_d[:, :],
                in_offset=IndirectOffsetOnAxis(ap=nodeidx_sb[:, w : w + 1], axis=0),
            )
            kw = K[w]
            for j in range(kw):
                c = cbase[w] + j
                g = sb.tile([P, 33], dtype=bf16, tag="g")
                nc.gpsimd.indirect_dma_start(
                    out=g[:], out_offset=None, in_=hp2_d[:, :],
                    in_offset=IndirectOffsetOnAxis(ap=srct_sb[:, c : c + 1], axis=0),
                )
                S = sb.tile([P, P], dtype=bf16, tag="S")
                nc.vector.tensor_tensor(
                    out=S[:], in0=dstlt_sb[:, c : c + 1].to_broadcast([P, P]),
                    in1=iota_sb[:], op=OP.is_equal,
                )
                stp = psstp.tile([P, P], dtype=bf16, tag="stp")
                nc.tensor.transpose(stp[:], S[:], ident_bf[:])
                St = sb.tile([P, P], dtype=bf16, tag="St")
                nc.vector.tensor_copy(St[:], stp[:])
                edp = psed.tile([P, 1], dtype=f32, tag="ed")
                nc.tensor.matmul(out=edp[:], lhsT=St[:], rhs=adw[:],
                                 start=True, stop=True)
                e1 = sbs.tile([P, 1], dtype=f32, tag="e1")
                nc.vector.tensor_tensor(
                    out=e1[:], in0=g[:, 32:33], in1=edp[:], op=OP.add
                )
                el = sbs.tile([P, 1], dtype=f32, tag="el")
                nc.scalar.activation(el[:], e1[:], FT.Prelu, alpha=0.2)
                V = sb.tile([P, 33], dtype=bf16, tag="V")
                nc.scalar.activation(V[:, 32:33], el[:], FT.Exp)
                nc.vector.tensor_tensor(
                    out=V[:, 0:HID], in0=g[:, 0:HID],
                    in1=V[:, 32:33].to_broadcast([P, HID]), op=OP.mult,
                )
                nc.tensor.matmul(
                    out=win_ps[:], lhsT=S[:], rhs=V[:],
                    start=(j == 0), stop=(j == kw - 1),
                )

            den = sbe.tile([P, 1], dtype=f32, tag="den")
            nc.vector.tensor_scalar(
                out=den[:], in0=win_ps[:, 32:33], scalar1=1e-30,
                scalar2=None, op0=OP.max,
            )
            rec = sbe.tile([P, 1], dtype=f32, tag="rec")
            nc.vector.reciprocal(rec[:], den[:])
            h2 = sbe.tile([P, HID], dtype=f32, tag="h2")
            nc.vector.tensor_tensor(
                out=h2[:], in0=win_ps[:, 0:HID],
                in1=rec[:].to_broadcast([P, HID]), op=OP.mult,
            )
            nc.vector.tensor_tensor(out=h2[:], in0=h2[:], in1=b2b_sb[:], op=OP.add)
            he = sbe.tile([P, HID], dtype=f32, tag="he")
            _elu(nc, sbe, he[:], h2[:], "e2")
            nc.tensor.matmul(
                out=pool_ps[:], lhsT=gon_sb[:, w * NGRAPH : (w + 1) * NGRAPH],
                rhs=he[:], start=(w == 0), stop=(w == NWIN - 1),
            )
        po = const.tile([NGRAPH, HID], dtype=f32)
        nc.vector.tensor_copy(po[:], pool_ps[:])
        nc.sync.dma_start(out=outb_d[:, :], in_=po[:, :])
    nc.compile()
    return nc


# ---------------------------------------------------------------- launch C
def _build_C():
    nc = bacc.Bacc("TRN2", target_bir_lowering=False, debug=False, num_devices=1)
    part_d = nc.dram_tensor("part", [NGRAPH, NCORES * HID], f32, kind="ExternalInput")
    invc_d = nc.dram_tensor("invc", [NGRAPH, 1], f32, kind="ExternalInput")
    fcw_d = nc.dram_tensor("fcw", [HID, NCLS], f32, kind="ExternalInput")
    fcbb_d = nc.dram_tensor("fcbb", [NGRAPH, NCLS], f32, kind="ExternalInput")
    out_d = nc.dram_tensor("out", [NGRAPH, NCLS], f32, kind="ExternalOutput")

    with tile.TileContext(nc, num_cores=1) as tc, ExitStack() as ctx:
        sb = ctx.enter_context(tc.tile_pool(name="sb", bufs=1))
        ps = ctx.enter_context(tc.tile_pool(name="ps", bufs=1, space="PSUM"))
        ident = sb.tile([NGRAPH, NGRAPH], dtype=f32)
        make_identity(nc, ident[:])
        pa = sb.tile([NGRAPH, NCORES * HID], dtype=f32)
        nc.sync.dma_start(out=pa[:], in_=part_d[:, :])
        invc = sb.tile([NGRAPH, 1], dtype=f32)
        nc.sync.dma_start(out=invc[:], in_=invc_d[:, :])
        fcw = sb.tile([HID, NCLS], dtype=f32)
        nc.sync.dma_start(out=fcw[:], in_=fcw_d[:, :])
        fcbb = sb.tile([NGRAPH, NCLS], dtype=f32)
        nc.sync.dma_start(out=fcbb[:], in_=fcbb_d[:, :])

        acc = sb.tile([NGRAPH, HID], dtype=f32)
        nc.vector.tensor_copy(acc[:], pa[:, 0:HID])
        for k in range(1, NCORES):
            nc.vector.tensor_tensor(
                out=acc[:], in0=acc[:], in1=pa[:, k * HID : (k + 1) * HID], op=OP.add
            )
        nc.vector.tensor_tensor(
            out=acc[:], in0=acc[:], in1=invc[:].to_broadcast([NGRAPH, HID]),
            op=OP.mult,
        )
        tps = ps.tile([HID, NGRAPH], dtype=f32)
        nc.tensor.transpose(tps[:], acc[:], ident[:])
        gT = sb.tile([HID, NGRAPH], dtype=f32)
        nc.scalar.copy(gT[:], tps[:])
        ops = ps.tile([NGRAPH, NCLS], dtype=f32)
        nc.tensor.matmul(out=ops[:], lhsT=gT[:], rhs=fcw[:], start=True, stop=True)
        osb = sb.tile([NGRAPH, NCLS], dtype=f32)
        nc.vector.tensor_tensor(out=osb[:], in0=ops[:], in1=fcbb[:], op=OP.add)
        nc.sync.dma_start(out=out_d[:, :], in_=osb[:, :])
    nc.compile()
    return nc


# ---------------------------------------------------------------- driver
def _run(inputs, trace=False):
    x = np.ascontiguousarray(np.asarray(inputs["x"], dtype=np.float32))
    edge_index = np.asarray(inputs["edge_index"], dtype=np.int64)
    batch = np.asarray(inputs["batch"], dtype=np.int64)
    W1 = np.ascontiguousarray(np.asarray(inputs["W1"], dtype=np.float32))
    att_src1 = np.asarray(inputs["att_src1"], dtype=np.float32)
    att_dst1 = np.asarray(inputs["att_dst1"], dtype=np.float32)
    b1 = np.asarray(inputs["b1"], dtype=np.float32)
    W2 = np.ascontiguousarray(np.asarray(inputs["W2"], dtype=np.float32))
    att_src2 = np.asarray(inputs["att_src2"], dtype=np.float32)
    att_dst2 = np.asarray(inputs["att_dst2"], dtype=np.float32)
    b2 = np.asarray(inputs["b2"], dtype=np.float32)
    fc_w = np.ascontiguousarray(np.asarray(inputs["fc_w"], dtype=np.float32))
    fc_b = np.asarray(inputs["fc_b"], dtype=np.float32)

    key = (edge_index.tobytes(), batch.tobytes())
    hkey = hash(key)
    if hkey not in _cache:
        prep = _host_prep(edge_index, batch)
        K, nchunk, cbase = prep[0], prep[1], prep[2]
        ncA = _build_A(K, nchunk, cbase)
        ncB = _build_B(K, nchunk, cbase)
        ncC = _build_C()
        _cache.clear()
        _cache[hkey] = (prep, ncA, ncB, ncC)
    prep, ncA, ncB, ncC = _cache[hkey]
    K, nchunk, cbase, SRCT, DSTIT, DSTLT, GON, INVC = prep
    NODEIDX = np.zeros((NCORES, P, NWIN), dtype=np.int32)
    for k in range(NCORES):
        for w in range(NWIN):
            NODEIDX[k, :, w] = np.minimum(k * NPC + w * P + np.arange(P), N_NODES - 1)
    NODEIDX = [np.ascontiguousarray(NODEIDX[k]) for k in range(NCORES)]

    # layout-only host tensors
    acat = np.zeros((2, P, 16), dtype=np.float32)  # [halfrow, f, 16]
    for h in range(HEADS):
        for c in range(HID):
            gidx = h * HID + c
            acat[gidx // P, gidx % P, h] = att_src1[h, c]
            acat[gidx // P, gidx % P, 8 + h] = att_dst1[h, c]
    acat = np.ascontiguousarray(acat.transpose(1, 0, 2))  # [P, 2, 16]
    iota = np.tile(np.arange(P, dtype=np.float32), (P, 1))
    b1b = np.tile(b1.reshape(1, HH), (P, 1)).astype(np.float32)
    w2r = np.concatenate([W2[0:P, :], W2[P : 2 * P, :]], axis=1)  # [128, 64]
    att2 = np.stack([att_src2[0], att_dst2[0]], axis=1).astype(np.float32)  # [32,2]
    b2b = np.tile(b2.reshape(1, HID), (P, 1)).astype(np.float32)
    fcbb = np.tile(fc_b.reshape(1, NCLS), (NGRAPH, 1)).astype(np.float32)

    in_maps_A = [
        {
            "x": x, "w1": W1, "acat": acat, "iota": iota, "b1b": b1b,
            "w2r": w2r, "att2": att2,
            "srct": np.ascontiguousarray(SRCT[k]),
            "nodeidx": NODEIDX[k],
            "dstlt": np.ascontiguousarray(DSTLT[k]),
        }
        for k in range(NCORES)
    ]
    resA = run_bass_kernel_spmd(ncA, in_maps_A, list(range(NCORES)), trace=trace)
    outAs = [resA.results[k]["outA"] for k in range(NCORES)]
    hp2 = np.ascontiguousarray(
        np.concatenate([o[:, 0:33] for o in outAs], axis=0).astype(ml_dtypes.bfloat16)
    )
    ad2 = np.ascontiguousarray(
        np.concatenate([o[:, 33:34] for o in outAs], axis=0).astype(ml_dtypes.bfloat16)
    )

    in_maps_B = [
        {
            "hp2": hp2, "ad2": ad2, "iota": iota, "b2b": b2b,
            "gon": np.ascontiguousarray(GON[k]),
            "srct": np.ascontiguousarray(SRCT[k]),
            "nodeidx": NODEIDX[k],
            "dstlt": np.ascontiguousarray(DSTLT[k]),
        }
        for k in range(NCORES)
    ]
    resB = run_bass_kernel_spmd(ncB, in_maps_B, list(range(NCORES)), trace=trace)
    part = np.concatenate(
        [resB.results[k]["outB"] for k in range(NCORES)], axis=1
    )  # [64, 8*32]
    in_map_C = {
        "part": np.ascontiguousarray(part), "invc": INVC, "fcw": fc_w, "fcbb": fcbb,
    }
    resC = run_bass_kernel_spmd(ncC, [in_map_C], [0], trace=trace)
    out = resC.results[0]["out"]
    return out, (resA, resB, resC)


def kernel(**inputs) -> np.ndarray:
    out, _ = _run(inputs, trace=False)
    return out


def run_profiled(inputs):
    return _run(inputs, trace=True)
